# revision 4
# baseline (speedup 1.0000x reference)
"""CARAFE-downsample (K=5, stride=2) Trainium2 kernel, 8-core SPMD.

Key ideas:
- Host de-interleaves x into 4 parity subgrids (fp16) so every
  stride-2 access (conv3x3 taps + all 25 reassembly taps) becomes a
  contiguous slice.
- Both 128-channel chips live in ONE SBUF tile (chip dim inside the
  free dims), so each reassembly mult/add is a single FD=2048 DVE op
  instead of two FD=1024 ops: 49 ops/chunk instead of 98, amortizing
  the ~160ns/op DVE overhead.
- conv+softmax mask production is split into 4 row-slabs (8 output
  rows each) and issued ahead of reassembly (A0 A1 R0 A2 A3 R1), so
  the serial head is only two slabs deep and chunk-1 masks are
  produced while chunk-0 reassembly runs.
- Masks bounce through DRAM per half-chunk and are broadcast to 128
  partitions by replicating DMAs; mb pool slots auto-throttle the
  prefetch of chunk-1 masks during chunk-0 reassembly.
- Reassembly adds use two independent accumulator chains to avoid
  back-to-back RAW stalls on the DVE.
- Output stays fp16 and is DMA'd straight from the accumulator
  (host upcasts); mask-softmax path stays fp32 (the 16-way channel
  product amplifies logit errors).
Sharding: core = batch*2 + H-half; zero-padded 2-row/2-col halo.
"""

import numpy as np

import concourse.bacc as bacc
import concourse.mybir as mybir
import concourse.tile as tile

F32 = mybir.dt.float32
F16 = mybir.dt.float16
AX = mybir.AxisListType
OP = mybir.AluOpType
ACTF = mybir.ActivationFunctionType

C, CC, H, W = 256, 64, 128, 128
B = 4
HO, WO = 32, 64           # per-core output dims
NPOS = HO * WO            # 2048
K5 = 5
GH, GW = 34, 66           # subgrid dims (rows, cols)
GSZ = GH * GW             # 2244 flat
NPC = 2                   # reassembly chunks (16 rows / 1024 pos)
PC = NPOS // NPC          # 1024 positions/chunk
NSLAB = 4                 # conv/mask slabs (8 output rows each)
SPC = NPOS // NSLAB       # 512 positions/slab
TPS = 4                   # 128-pos tiles per slab

# cx subgrid row ranges per slab: block c4 reads rows [8*c4, 8*c4+10)
SLAB_ROWS = [(0, 10), (10, 18), (18, 26), (26, 34)]


def build_nc():
    nc = bacc.Bacc("TRN2", target_bir_lowering=False, debug=False)

    # inputs
    xq = nc.dram_tensor("xq", [C, 4, GH, GW], F16, kind="ExternalInput")
    w2a = nc.dram_tensor("w2a", [128, 128], F16, kind="ExternalInput")
    w2b = nc.dram_tensor("w2b", [128, 128], F16, kind="ExternalInput")
    wp = nc.dram_tensor("wp", [128, 3 * 41], F32, kind="ExternalInput")
    ws = nc.dram_tensor("ws", [64, 3 * 41], F32, kind="ExternalInput")
    ident = nc.dram_tensor("ident", [128, 128], F32, kind="ExternalInput")
    mscr = nc.dram_tensor("mscr", [25, NPOS], F16)
    y = nc.dram_tensor("y", [C, HO, WO], F16, kind="ExternalOutput")

    # conv3x3 tap schedule: 3 pairs (contract 128) + 3 singles (64)
    PAIRS = [(3, 0, 0), (3, 1, 0), (1, 1, 0)]   # (g, dh, dw)
    SINGLES = [(2, 0, 1), (2, 1, 1), (0, 1, 1)]

    with tile.TileContext(nc) as tc:
        with (
            tc.tile_pool(name="big", bufs=1) as bigpool,
            tc.tile_pool(name="work", bufs=3) as workpool,
            tc.tile_pool(name="tmp", bufs=4) as tmppool,
            tc.tile_pool(name="acb", bufs=2) as accbpool,
            tc.tile_pool(name="mbp", bufs=27) as mbpool,
            tc.tile_pool(name="ps", bufs=2, space="PSUM") as ps,
            tc.tile_pool(name="ps2", bufs=2, space="PSUM") as ps2,
            tc.tile_pool(name="ps3", bufs=2, space="PSUM") as ps3,
        ):
            # ---- persistent tiles ----
            xboth = bigpool.tile([128, 2, 4, GH, GW], F16, tag="xboth")
            w2as = bigpool.tile([128, 128], F16, tag="w2a")
            w2bs = bigpool.tile([128, 128], F16, tag="w2b")
            wps = bigpool.tile([128, 3 * 41], F32, tag="wp")
            wss = bigpool.tile([64, 3 * 41], F32, tag="ws")
            ids = bigpool.tile([128, 128], F32, tag="ident")
            cxd = bigpool.tile([128, 4, GH, GW], F32, tag="cxd")
            logits = bigpool.tile([41, NPOS], F32, tag="logits")
            mcm = bigpool.tile([25, NPOS], F16, tag="mcm")
            accboth = bigpool.tile([128, 2, HO, WO], F16, tag="acc")
            ewarm = bigpool.tile([1, 1], F32, tag="ewarm")

            nc.sync.dma_start(out=w2as[:], in_=w2a[:])
            nc.sync.dma_start(out=w2bs[:], in_=w2b[:])
            nc.sync.dma_start(out=wps[:], in_=wp[:])
            nc.sync.dma_start(out=wss[:], in_=ws[:])
            nc.sync.dma_start(out=ids[:], in_=ident[:])
            # pre-warm the exp activation table during the x DMAs
            nc.scalar.activation(ewarm[:], ids[0:1, 0:1], ACTF.Exp)

            # x loads split by (slab, subgrid, chip) so conv starts early
            for (sa, sb) in SLAB_ROWS:
                for g in range(4):
                    nc.sync.dma_start(out=xboth[:, 0, g, sa:sb],
                                      in_=xq[0:128, g, sa:sb])
                    nc.sync.dma_start(out=xboth[:, 1, g, sa:sb],
                                      in_=xq[128:256, g, sa:sb])

            cxf = cxd.rearrange("p g h w -> p (g h w)")
            xf = xboth.rearrange("p t g h w -> p t (g h w)")
            yf = y.rearrange("c h w -> c (h w)")

            mbs = {}

            def emit_slab(s):
                ra, rb = SLAB_ROWS[s]
                # ---- conv1x1 on this slab ----
                for g in range(4):
                    o0 = g * GSZ + ra * GW
                    n = (rb - ra) * GW
                    m1 = ((n // 2) + 1) & ~1
                    for (s0, m) in ((0, m1), (m1, n - m1)):
                        pt = ps.tile([128, 512], F32, tag="ps1")
                        nc.tensor.matmul(pt[:, 0:m], w2as[:],
                                         xf[:, 0, o0 + s0:o0 + s0 + m],
                                         start=True, stop=False)
                        nc.tensor.matmul(pt[:, 0:m], w2bs[:],
                                         xf[:, 1, o0 + s0:o0 + s0 + m],
                                         start=False, stop=True)
                        nc.scalar.activation(cxf[0:64, o0 + s0:o0 + s0 + m],
                                             pt[0:64, 0:m], ACTF.Copy)
                        if g in (1, 3):
                            # shifted bank: dst[i] = cx[i+1]
                            d0 = o0 + s0 - 1
                            if d0 < g * GSZ:
                                nc.scalar.activation(
                                    cxf[64:128, d0 + 1:d0 + m],
                                    pt[64:128, 1:m], ACTF.Copy)
                            else:
                                nc.scalar.activation(
                                    cxf[64:128, d0:d0 + m],
                                    pt[64:128, 0:m], ACTF.Copy)

                # ---- conv3x3 (paired) -> logits for block c4 = s ----
                hoc = 8 * s
                lgp = ps2.tile([41, 512], F32, tag="ps2")
                nmm = 0
                for j, (g, dh, dw) in enumerate(PAIRS):
                    rhs = cxd[:, g, hoc + dh: hoc + dh + 8, dw: dw + 64]
                    nc.tensor.matmul(lgp[:], wps[:, 41 * j: 41 * (j + 1)],
                                     rhs, start=(nmm == 0), stop=False)
                    nmm += 1
                for j, (g, dh, dw) in enumerate(SINGLES):
                    rhs = cxd[0:64, g, hoc + dh: hoc + dh + 8, dw: dw + 64]
                    nc.tensor.matmul(lgp[:], wss[:, 41 * j: 41 * (j + 1)],
                                     rhs, start=False, stop=(j == 2))
                nc.scalar.activation(logits[:, 512 * s: 512 * (s + 1)],
                                     lgp[:], ACTF.Copy)

                # ---- transpose logits -> pos-major ----
                lgT = workpool.tile([128, TPS, 41], F32, tag="lgT")
                for tt in range(TPS):
                    t = TPS * s + tt
                    tpp = ps3.tile([128, 41], F32, tag="ps3")
                    nc.tensor.transpose(tpp[:],
                                        logits[:, 128 * t: 128 * (t + 1)],
                                        ids[0:41, 0:41])
                    nc.scalar.activation(lgT[:, tt, :], tpp[:], ACTF.Copy)

                # ---- mask pipeline (pos-major) ----
                p8 = workpool.tile([128, TPS, 8], F32, tag="p8")
                nc.vector.tensor_tensor(p8[:], lgT[:, :, 25:33],
                                        lgT[:, :, 33:41], OP.mult)
                p4 = workpool.tile([128, TPS, 4], F32, tag="p4")
                nc.vector.tensor_tensor(p4[:], p8[:, :, 0:4], p8[:, :, 4:8],
                                        OP.mult)
                p2 = workpool.tile([128, TPS, 2], F32, tag="p2")
                nc.vector.tensor_tensor(p2[:], p4[:, :, 0:2], p4[:, :, 2:4],
                                        OP.mult)
                i0 = workpool.tile([128, TPS], F32, tag="i0")
                nc.vector.tensor_tensor(i0[:], p2[:, :, 0], p2[:, :, 1],
                                        OP.mult)
                ic = workpool.tile([128, TPS], F32, tag="ic")
                nc.vector.tensor_scalar(ic[:], i0[:], 10.0, -10.0,
                                        OP.min, OP.max)

                mskl = workpool.tile([128, TPS, 25], F32, tag="mskl")
                nc.vector.tensor_tensor(mskl[:], lgT[:, :, 0:25],
                                        ic[:].to_broadcast([128, TPS, 25]),
                                        OP.mult)
                tmax = workpool.tile([128, TPS], F32, tag="tmax")
                nc.vector.tensor_reduce(tmax[:], mskl[:], AX.X, OP.max)
                msub = workpool.tile([128, TPS, 25], F32, tag="msub")
                nc.vector.tensor_tensor(msub[:], mskl[:],
                                        tmax[:].to_broadcast([128, TPS, 25]),
                                        OP.subtract)
                mexp = workpool.tile([128, TPS, 25], F32, tag="mexp")
                nc.scalar.activation(mexp[:], msub[:], ACTF.Exp)
                msum = workpool.tile([128, TPS], F32, tag="msum")
                nc.vector.tensor_reduce(msum[:], mexp[:], AX.X, OP.add)
                mrec = workpool.tile([128, TPS], F32, tag="mrec")
                nc.vector.reciprocal(mrec[:], msum[:])
                mskn = workpool.tile([128, TPS, 25], F32, tag="mskn")
                nc.vector.tensor_tensor(mskn[:], mexp[:],
                                        mrec[:].to_broadcast([128, TPS, 25]),
                                        OP.mult)

                # ---- transpose mask back to channel-major (fp16) ----
                for tt in range(TPS):
                    t = TPS * s + tt
                    mcp = ps3.tile([25, 128], F32, tag="ps3")
                    nc.tensor.transpose(mcp[:], mskn[:, tt, :], ids[:])
                    nc.scalar.activation(mcm[:, 128 * t: 128 * (t + 1)],
                                         mcp[:], ACTF.Copy)

                # ---- bounce mask slab to DRAM for replicating DMAs ----
                nc.sync.dma_start(out=mscr[:, SPC * s: SPC * (s + 1)],
                                  in_=mcm[:, SPC * s: SPC * (s + 1)])

            def emit_broadcast(pc):
                lst = []
                for k in range(K5 * K5):
                    mb = mbpool.tile([128, PC], F16, tag="mb")
                    nc.sync.dma_start(
                        out=mb[:],
                        in_=mscr[k: k + 1,
                                 PC * pc: PC * (pc + 1)].to_broadcast(
                                     [128, PC]))
                    lst.append(mb)
                mbs[pc] = lst

            def tap_aps(pc, k):
                ho0 = 16 * pc
                ky, kx = k // K5, k % K5
                g = 2 * (ky % 2) + (kx % 2)
                xsrc = xboth[:, :, g, ho0 + ky // 2: ho0 + ky // 2 + 16,
                             kx // 2: kx // 2 + 64]
                mbv = mbs[pc][k].rearrange(
                    "p (o h w) -> p o h w", o=1, h=16,
                    w=64).to_broadcast([128, 2, 16, 64])
                return xsrc, mbv

            def emit_reassembly(pc):
                ho0 = 16 * pc
                accv = accboth[:, :, ho0: ho0 + 16, :]
                accb = accbpool.tile([128, 2, 16, 64], F16, tag="accb")
                chain = {0: accv, 1: accb[:]}
                tmps = {}
                # taps 0/1 seed the two chains, then mult i / add i-2
                xs0, mb0 = tap_aps(pc, 0)
                nc.vector.tensor_tensor(accv, xs0, mb0, OP.mult)
                xs1, mb1 = tap_aps(pc, 1)
                nc.vector.tensor_tensor(accb[:], xs1, mb1, OP.mult)
                for i in range(2, 27):
                    if i < 25:
                        xsi, mbi = tap_aps(pc, i)
                        t = tmppool.tile([128, 2, 16, 64], F16, tag="tp")
                        nc.vector.tensor_tensor(t[:], xsi, mbi, OP.mult)
                        tmps[i] = t
                    j = i - 2
                    if j >= 2:
                        dst = chain[j % 2]
                        nc.vector.tensor_tensor(dst, dst, tmps.pop(j)[:],
                                                OP.add)
                nc.vector.tensor_tensor(accv, accv, accb[:], OP.add)

                # ---- store this chunk (fp16, host upcasts) ----
                for ch in range(2):
                    nc.scalar.dma_start(
                        out=yf[128 * ch: 128 * (ch + 1),
                               PC * pc: PC * (pc + 1)],
                        in_=accboth[:, ch, ho0: ho0 + 16, :].rearrange(
                            "p h w -> p (h w)"))

            # ---- pipeline: A0 A1 [bc0] R0 A2 A3 [bc1] R1 ----
            emit_slab(0)
            emit_slab(1)
            emit_broadcast(0)
            emit_slab(2)
            emit_slab(3)
            emit_broadcast(1)
            emit_reassembly(0)
            emit_reassembly(1)

    nc.finalize()
    return nc


def make_core_inputs(x, w_comp, b_comp, w_enc, b_enc, w_kenc, b_kenc):
    """Full inputs -> list of 8 per-core input dicts."""
    x = np.asarray(x)
    w_compT = np.ascontiguousarray(
        np.asarray(w_comp).reshape(CC, C).T).astype(np.float32)  # [256, 64]
    # conv1x1 stationaries: [w | w] duplicated out-cols, fp16
    w2a = np.concatenate([w_compT[0:128]] * 2, axis=1).astype(np.float16)
    w2b = np.concatenate([w_compT[128:256]] * 2, axis=1).astype(np.float16)

    we = np.asarray(w_enc)    # [25, 64, 3, 3]
    wk = np.asarray(w_kenc)   # [16, 64, 3, 3]
    w41 = np.concatenate([we, wk], axis=0)  # [41, 64, 3, 3]

    # pair stationaries [128, 3*41]; singles [64, 3*41]
    PAIR_TAPS = [((0, 0), (0, 2)), ((2, 0), (2, 2)), ((1, 0), (1, 2))]
    SINGLE_TAPS = [(0, 1), (2, 1), (1, 1)]
    wp = np.zeros((128, 3, 41), np.float32)
    for j, (ta, tb) in enumerate(PAIR_TAPS):
        wp[0:64, j] = w41[:, :, ta[0], ta[1]].T
        wp[64:128, j] = w41[:, :, tb[0], tb[1]].T
    wp = wp.reshape(128, 3 * 41)
    wss = np.zeros((64, 3, 41), np.float32)
    for j, (dy, dx) in enumerate(SINGLE_TAPS):
        wss[:, j] = w41[:, :, dy, dx].T
    wss = wss.reshape(64, 3 * 41)
    ident = np.eye(128, dtype=np.float32)

    maps = []
    for core in range(8):
        b, h = core // 2, core % 2
        start = 64 * h
        xpc = np.zeros((C, 68, 132), np.float32)
        lo, hi = start - 2, start + 66
        clo, chi = max(lo, 0), min(hi, H)
        xpc[:, clo - lo: clo - lo + (chi - clo), 2:130] = x[b, :, clo:chi, :]
        # de-interleave: g = 2*(row%2) + (col%2)
        xqc = np.empty((C, 4, GH, GW), np.float16)
        xqc[:, 0] = xpc[:, 0::2, 0::2]
        xqc[:, 1] = xpc[:, 0::2, 1::2]
        xqc[:, 2] = xpc[:, 1::2, 0::2]
        xqc[:, 3] = xpc[:, 1::2, 1::2]
        maps.append({
            "xq": xqc,
            "w2a": w2a,
            "w2b": w2b,
            "wp": wp,
            "ws": wss,
            "ident": ident,
        })
    return maps


def assemble_output(results):
    out = np.zeros((B, C, 64, 64), np.float32)
    for core in range(8):
        b, h = core // 2, core % 2
        out[b, :, 32 * h: 32 * (h + 1), :] = results[core]["y"].astype(
            np.float32)
    return out


_NC_CACHE = []


def kernel(**inputs):
    import numpy as _np
    from concourse.bass_utils import run_bass_kernel_spmd

    maps = make_core_inputs(
        inputs["x"], inputs["w_comp"], inputs["b_comp"], inputs["w_enc"],
        inputs["b_enc"], inputs["w_kenc"], inputs["b_kenc"])
    if not _NC_CACHE:
        _NC_CACHE.append(build_nc())
    res = run_bass_kernel_spmd(_NC_CACHE[0], maps, list(range(8)))
    out = assemble_output(res.results)
    return out.astype(_np.float32)


# revision 12
# speedup vs baseline: 1.2152x; 1.2152x over previous
"""CARAFE-downsample (K=5, stride=2) Trainium2 kernel, 8-core SPMD.

Key ideas:
- Host de-interleaves x into 4 parity subgrids (fp16) so every
  stride-2 access (conv3x3 taps + all 25 reassembly taps) becomes a
  contiguous slice.
- Both 128-channel chips live in ONE SBUF tile (chip dim inside the
  free dims), so each reassembly mult/add is a single FD=2048 DVE op
  instead of two FD=1024 ops: 49 ops/chunk instead of 98, amortizing
  the ~160ns/op DVE overhead.
- conv+softmax mask production is split into 4 row-slabs (8 output
  rows each) and issued ahead of reassembly (A0 A1 R0 A2 A3 R1), so
  the serial head is only two slabs deep and chunk-1 masks are
  produced while chunk-0 reassembly runs.
- Masks bounce through DRAM per half-chunk and are broadcast to 128
  partitions by replicating DMAs; mb pool slots auto-throttle the
  prefetch of chunk-1 masks during chunk-0 reassembly.
- Reassembly adds use two independent accumulator chains to avoid
  back-to-back RAW stalls on the DVE.
- Output stays fp16 and is DMA'd straight from the accumulator
  (host upcasts); mask-softmax path stays fp32 (the 16-way channel
  product amplifies logit errors).
Sharding: core = batch*2 + H-half; zero-padded 2-row/2-col halo.
"""

import numpy as np

import concourse.bacc as bacc
import concourse.mybir as mybir
import concourse.tile as tile

F32 = mybir.dt.float32
F16 = mybir.dt.float16
AX = mybir.AxisListType
OP = mybir.AluOpType
ACTF = mybir.ActivationFunctionType

C, CC, H, W = 256, 64, 128, 128
B = 4
HO, WO = 32, 64           # per-core output dims
NPOS = HO * WO            # 2048
K5 = 5
GH, GW = 34, 66           # subgrid dims (rows, cols)
GSZ = GH * GW             # 2244 flat
NPC = 2                   # reassembly chunks (16 rows / 1024 pos)
PC = NPOS // NPC          # 1024 positions/chunk
NSLAB = 4                 # conv/mask slabs (8 output rows each)
SPC = NPOS // NSLAB       # 512 positions/slab
TPS = 4                   # 128-pos tiles per slab

# cx subgrid row ranges per slab: block c4 reads rows [8*c4, 8*c4+10)
SLAB_ROWS = [(0, 10), (10, 18), (18, 26), (26, 34)]


def build_nc():
    nc = bacc.Bacc("TRN2", target_bir_lowering=False, debug=False)

    # inputs
    xq = nc.dram_tensor("xq", [C, 4, GH, GW], F16, kind="ExternalInput")
    w2a = nc.dram_tensor("w2a", [128, 128], F16, kind="ExternalInput")
    w2b = nc.dram_tensor("w2b", [128, 128], F16, kind="ExternalInput")
    wp = nc.dram_tensor("wp", [128, 3 * 41], F16, kind="ExternalInput")
    ws = nc.dram_tensor("ws", [64, 3 * 41], F16, kind="ExternalInput")
    ident = nc.dram_tensor("ident", [128, 128], F32, kind="ExternalInput")
    mscr = nc.dram_tensor("mscr", [25, NPOS], F16)
    y = nc.dram_tensor("y", [C, HO, WO], F16, kind="ExternalOutput")

    # conv3x3 tap schedule: 3 pairs (contract 128) + 3 singles (64)
    PAIRS = [(3, 0, 0), (3, 1, 0), (1, 1, 0)]   # (g, dh, dw)
    SINGLES = [(2, 0, 1), (2, 1, 1), (0, 1, 1)]

    with tile.TileContext(nc) as tc:
        with (
            tc.tile_pool(name="big", bufs=1) as bigpool,
            tc.tile_pool(name="work", bufs=3) as workpool,
            tc.tile_pool(name="tmp", bufs=6) as tmppool,
            tc.tile_pool(name="mbp", bufs=27) as mbpool,
            tc.tile_pool(name="ps", bufs=2, space="PSUM") as ps,
            tc.tile_pool(name="ps2", bufs=2, space="PSUM") as ps2,
            tc.tile_pool(name="ps3", bufs=2, space="PSUM") as ps3,
            tc.tile_pool(name="ps4", bufs=2, space="PSUM") as ps4,
        ):
            # ---- persistent tiles ----
            xboth = bigpool.tile([128, 2, 4, GH, GW], F16, tag="xboth")
            w2as = bigpool.tile([128, 128], F16, tag="w2a")
            w2bs = bigpool.tile([128, 128], F16, tag="w2b")
            wps = bigpool.tile([128, 3 * 41], F16, tag="wp")
            wss = bigpool.tile([64, 3 * 41], F16, tag="ws")
            ids = bigpool.tile([128, 128], F32, tag="ident")
            cxd = bigpool.tile([128, 4, GH, GW], F16, tag="cxd")
            logits = bigpool.tile([41, NPOS], F32, tag="logits")
            mcm = bigpool.tile([25, NPOS], F16, tag="mcm")
            accboth = bigpool.tile([128, 2, HO, WO], F16, tag="acc")
            ewarm = bigpool.tile([1, 1], F32, tag="ewarm")

            nc.sync.dma_start(out=w2as[:], in_=w2a[:])
            nc.sync.dma_start(out=w2bs[:], in_=w2b[:])
            nc.sync.dma_start(out=wps[:], in_=wp[:])
            nc.sync.dma_start(out=wss[:], in_=ws[:])
            nc.sync.dma_start(out=ids[:], in_=ident[:])
            # pre-warm the exp activation table during the x DMAs
            nc.scalar.activation(ewarm[:], ids[0:1, 0:1], ACTF.Exp)

            # x loads split by (slab, subgrid, chip) so conv starts early
            for (sa, sb) in SLAB_ROWS:
                for g in range(4):
                    nc.sync.dma_start(out=xboth[:, 0, g, sa:sb],
                                      in_=xq[0:128, g, sa:sb])
                    nc.sync.dma_start(out=xboth[:, 1, g, sa:sb],
                                      in_=xq[128:256, g, sa:sb])

            cxf = cxd.rearrange("p g h w -> p (g h w)")
            xf = xboth.rearrange("p t g h w -> p t (g h w)")
            yf = y.rearrange("c h w -> c (h w)")

            mbs = {}

            def emit_slab(s):
                ra, rb = SLAB_ROWS[s]
                # ---- conv1x1 on this slab ----
                for g in range(4):
                    o0 = g * GSZ + ra * GW
                    n = (rb - ra) * GW
                    m1 = ((n // 2) + 1) & ~1
                    for (s0, m) in ((0, m1), (m1, n - m1)):
                        pt = ps.tile([128, 512], F32, tag="ps1")
                        nc.tensor.matmul(pt[:, 0:m], w2as[:],
                                         xf[:, 0, o0 + s0:o0 + s0 + m],
                                         start=True, stop=False)
                        nc.tensor.matmul(pt[:, 0:m], w2bs[:],
                                         xf[:, 1, o0 + s0:o0 + s0 + m],
                                         start=False, stop=True)
                        nc.scalar.activation(cxf[0:64, o0 + s0:o0 + s0 + m],
                                             pt[0:64, 0:m], ACTF.Copy)
                        if g in (1, 3):
                            # shifted bank: dst[i] = cx[i+1]
                            d0 = o0 + s0 - 1
                            if d0 < g * GSZ:
                                nc.scalar.activation(
                                    cxf[64:128, d0 + 1:d0 + m],
                                    pt[64:128, 1:m], ACTF.Copy)
                            else:
                                nc.scalar.activation(
                                    cxf[64:128, d0:d0 + m],
                                    pt[64:128, 0:m], ACTF.Copy)

                # ---- conv3x3 (paired) -> logits for block c4 = s ----
                hoc = 8 * s
                lgp = ps2.tile([41, 512], F32, tag="ps2")
                nmm = 0
                for j, (g, dh, dw) in enumerate(PAIRS):
                    rhs = cxd[:, g, hoc + dh: hoc + dh + 8, dw: dw + 64]
                    nc.tensor.matmul(lgp[:], wps[:, 41 * j: 41 * (j + 1)],
                                     rhs, start=(nmm == 0), stop=False)
                    nmm += 1
                for j, (g, dh, dw) in enumerate(SINGLES):
                    rhs = cxd[0:64, g, hoc + dh: hoc + dh + 8, dw: dw + 64]
                    nc.tensor.matmul(lgp[:], wss[:, 41 * j: 41 * (j + 1)],
                                     rhs, start=False, stop=(j == 2))
                nc.scalar.activation(logits[:, 512 * s: 512 * (s + 1)],
                                     lgp[:], ACTF.Copy)

                # ---- transpose logits -> pos-major (one PSUM tile) ----
                lgTp = ps3.tile([128, TPS, 41], F32, tag="ps3")
                for tt in range(TPS):
                    t = TPS * s + tt
                    nc.tensor.transpose(lgTp[:, tt, :],
                                        logits[:, 128 * t: 128 * (t + 1)],
                                        ids[0:41, 0:41])
                lgT = workpool.tile([128, TPS, 41], F32, tag="lgT")
                nc.scalar.activation(lgT[:], lgTp[:], ACTF.Copy)

                # ---- mask pipeline (pos-major; exp is fp32-safe w/o max) --
                p8 = workpool.tile([128, TPS, 8], F32, tag="p8")
                nc.vector.tensor_tensor(p8[:], lgT[:, :, 25:33],
                                        lgT[:, :, 33:41], OP.mult)
                p4 = workpool.tile([128, TPS, 4], F32, tag="p4")
                nc.vector.tensor_tensor(p4[:], p8[:, :, 0:4], p8[:, :, 4:8],
                                        OP.mult)
                p2 = workpool.tile([128, TPS, 2], F32, tag="p2")
                nc.vector.tensor_tensor(p2[:], p4[:, :, 0:2], p4[:, :, 2:4],
                                        OP.mult)
                i0 = workpool.tile([128, TPS], F32, tag="i0")
                nc.vector.tensor_tensor(i0[:], p2[:, :, 0], p2[:, :, 1],
                                        OP.mult)
                ic = workpool.tile([128, TPS], F32, tag="ic")
                nc.vector.tensor_scalar(ic[:], i0[:], 10.0, -10.0,
                                        OP.min, OP.max)

                mskl = workpool.tile([128, TPS, 25], F32, tag="mskl")
                nc.vector.tensor_tensor(mskl[:], lgT[:, :, 0:25],
                                        ic[:].to_broadcast([128, TPS, 25]),
                                        OP.mult)
                mexp = workpool.tile([128, TPS, 25], F32, tag="mexp")
                nc.scalar.activation(mexp[:], mskl[:], ACTF.Exp)
                msum = workpool.tile([128, TPS], F32, tag="msum")
                nc.vector.tensor_reduce(msum[:], mexp[:], AX.X, OP.add)
                mrec = workpool.tile([128, TPS], F32, tag="mrec")
                nc.vector.reciprocal(mrec[:], msum[:])
                mskn = workpool.tile([128, TPS, 25], F32, tag="mskn")
                nc.vector.tensor_tensor(mskn[:], mexp[:],
                                        mrec[:].to_broadcast([128, TPS, 25]),
                                        OP.mult)

                # ---- transpose mask back to channel-major (fp16) ----
                for tt in range(TPS):
                    t = TPS * s + tt
                    mcp = ps4.tile([25, 128], F32, tag="ps4")
                    nc.tensor.transpose(mcp[:], mskn[:, tt, :], ids[:])
                    nc.scalar.activation(mcm[:, 128 * t: 128 * (t + 1)],
                                         mcp[:], ACTF.Copy)

                # ---- bounce mask slab to DRAM for replicating DMAs ----
                nc.sync.dma_start(out=mscr[:, SPC * s: SPC * (s + 1)],
                                  in_=mcm[:, SPC * s: SPC * (s + 1)])

            def emit_broadcast(pc):
                lst = []
                for k in range(K5 * K5):
                    mb = mbpool.tile([128, PC], F16, tag="mb")
                    nc.sync.dma_start(
                        out=mb[:],
                        in_=mscr[k: k + 1,
                                 PC * pc: PC * (pc + 1)].to_broadcast(
                                     [128, PC]))
                    lst.append(mb)
                mbs[pc] = lst

            def tap_aps(pc, k, ch):
                ho0 = 16 * pc
                ky, kx = k // K5, k % K5
                g = 2 * (ky % 2) + (kx % 2)
                xsrc = xboth[:, ch, g, ho0 + ky // 2: ho0 + ky // 2 + 16,
                             kx // 2: kx // 2 + 64]
                mbv = mbs[pc][k].rearrange("p (h w) -> p h w", h=16)
                return xsrc, mbv

            def emit_reassembly(pc):
                # unit u = (tap, chip); the two chip chains alternate so
                # consecutive DVE ops are independent
                ho0 = 16 * pc
                accv = [accboth[:, ch, ho0: ho0 + 16, :] for ch in range(2)]
                tmps = {}
                NU = 50
                for u in range(NU + 2):
                    if u < NU:
                        k, ch = u // 2, u % 2
                        xsrc, mbv = tap_aps(pc, k, ch)
                        if u < 2:
                            nc.vector.tensor_tensor(accv[ch], xsrc, mbv[:],
                                                    OP.mult)
                        else:
                            t = tmppool.tile([128, 16, 64], F16, tag="tp")
                            nc.vector.tensor_tensor(t[:], xsrc, mbv[:],
                                                    OP.mult)
                            tmps[u] = t
                    j = u - 2
                    if j >= 2 and j in tmps:
                        ch = j % 2
                        nc.vector.tensor_tensor(accv[ch], accv[ch],
                                                tmps.pop(j)[:], OP.add)

                # ---- store this chunk (fp16, host upcasts) ----
                for ch in range(2):
                    nc.scalar.dma_start(
                        out=yf[128 * ch: 128 * (ch + 1),
                               PC * pc: PC * (pc + 1)],
                        in_=accboth[:, ch, ho0: ho0 + 16, :].rearrange(
                            "p h w -> p (h w)"))

            # ---- pipeline: A0 A1 [bc0] R0 A2 A3 [bc1] R1 ----
            emit_slab(0)
            emit_slab(1)
            emit_broadcast(0)
            emit_slab(2)
            emit_slab(3)
            emit_broadcast(1)
            emit_reassembly(0)
            emit_reassembly(1)

    nc.finalize()
    return nc


def make_core_inputs(x, w_comp, b_comp, w_enc, b_enc, w_kenc, b_kenc):
    """Full inputs -> list of 8 per-core input dicts."""
    x = np.asarray(x)
    w_compT = np.ascontiguousarray(
        np.asarray(w_comp).reshape(CC, C).T).astype(np.float32)  # [256, 64]
    # conv1x1 stationaries: [w | w] duplicated out-cols, fp16
    w2a = np.concatenate([w_compT[0:128]] * 2, axis=1).astype(np.float16)
    w2b = np.concatenate([w_compT[128:256]] * 2, axis=1).astype(np.float16)

    we = np.asarray(w_enc)    # [25, 64, 3, 3]
    wk = np.asarray(w_kenc)   # [16, 64, 3, 3]
    w41 = np.concatenate([we, wk], axis=0)  # [41, 64, 3, 3]

    # pair stationaries [128, 3*41]; singles [64, 3*41]
    PAIR_TAPS = [((0, 0), (0, 2)), ((2, 0), (2, 2)), ((1, 0), (1, 2))]
    SINGLE_TAPS = [(0, 1), (2, 1), (1, 1)]
    wp = np.zeros((128, 3, 41), np.float32)
    for j, (ta, tb) in enumerate(PAIR_TAPS):
        wp[0:64, j] = w41[:, :, ta[0], ta[1]].T
        wp[64:128, j] = w41[:, :, tb[0], tb[1]].T
    wp = wp.reshape(128, 3 * 41).astype(np.float16)
    wss = np.zeros((64, 3, 41), np.float32)
    for j, (dy, dx) in enumerate(SINGLE_TAPS):
        wss[:, j] = w41[:, :, dy, dx].T
    wss = wss.reshape(64, 3 * 41).astype(np.float16)
    ident = np.eye(128, dtype=np.float32)

    maps = []
    for core in range(8):
        b, h = core // 2, core % 2
        start = 64 * h
        xpc = np.zeros((C, 68, 132), np.float32)
        lo, hi = start - 2, start + 66
        clo, chi = max(lo, 0), min(hi, H)
        xpc[:, clo - lo: clo - lo + (chi - clo), 2:130] = x[b, :, clo:chi, :]
        # de-interleave: g = 2*(row%2) + (col%2)
        xqc = np.empty((C, 4, GH, GW), np.float16)
        xqc[:, 0] = xpc[:, 0::2, 0::2]
        xqc[:, 1] = xpc[:, 0::2, 1::2]
        xqc[:, 2] = xpc[:, 1::2, 0::2]
        xqc[:, 3] = xpc[:, 1::2, 1::2]
        maps.append({
            "xq": xqc,
            "w2a": w2a,
            "w2b": w2b,
            "wp": wp,
            "ws": wss,
            "ident": ident,
        })
    return maps


def assemble_output(results):
    out = np.zeros((B, C, 64, 64), np.float32)
    for core in range(8):
        b, h = core // 2, core % 2
        out[b, :, 32 * h: 32 * (h + 1), :] = results[core]["y"].astype(
            np.float32)
    return out


_NC_CACHE = []


def kernel(**inputs):
    import numpy as _np
    from concourse.bass_utils import run_bass_kernel_spmd

    maps = make_core_inputs(
        inputs["x"], inputs["w_comp"], inputs["b_comp"], inputs["w_enc"],
        inputs["b_enc"], inputs["w_kenc"], inputs["b_kenc"])
    if not _NC_CACHE:
        _NC_CACHE.append(build_nc())
    res = run_bass_kernel_spmd(_NC_CACHE[0], maps, list(range(8)))
    out = assemble_output(res.results)
    return out.astype(_np.float32)
